# revision 56
# baseline (speedup 1.0000x reference)
"""Trainium2 Bass kernel for a ResNet BasicBlock (conv3x3-BN-conv3x3-+x-BN).

Full inputs -> full output. Internally: data-parallel over 8 NeuronCores on the
batch dim (32 images -> 4 per core), BN batch statistics all-reduced across
cores via a tiny AllReduce collective.

Device layout ("image-pair" scheme):
  - 2 images stacked on the 128 SBUF partitions: partition p = (img = p//64,
    ch = p%64).  Conv runs as 9 tap-matmuls (K=64, M=64, N=512) accumulated in
    PSUM, emitted tap-major alternating PE quadrants (img0 rows/cols 0-63,
    img1 rows/cols 64-127) so adjacent instructions overlap -> ~2 concurrent
    matmuls, the 50% ceiling for K=M=64 on the 128x128 array.
  - Spatial tiles are 4 interior rows x 128 cols = 512 columns (one PSUM bank,
    6-deep pool), so statistics accumulated at eviction never see the padding
    ring.  conv1 evicts psum->o1 on the scalar engine; conv2 evicts via a
    single vector add fusing the residual (o3 = psum + x).
  - BN stats: one vector bn_stats per eviction tile, one bn_aggr fold, then a
    1 KiB AllReduce of per-partition (mean, E[v^2]); partition halves are
    folded AFTER the collective by landing each DRAM half duplicated on all
    128 partitions (lane-aligned add, no cross-partition traffic).  A dummy
    AllReduce at kernel start pre-warms the collective stream.
  - BN coefficient chain is 6 ops deep: carries -mean so plain subtract works,
    and folds gamma into the invstd via Sqrt((-gamma^2) * (-1/(var+eps))).
  - BN1's affine is applied to o1 in-place in 8-row strips pipelined 1-2
    strips ahead of the conv2 tap reads (scalar engine works under the PE).
  - Output BN2-apply alternates scalar/vector engines over row stripes (small
    stripes first so the write DMA ramps early) and the store DMAs alternate
    between the sync and gpsimd queues.
  - Padding rings are zeroed with ring-only memsets; x is chunk-loaded (conv1
    tile t only waits for its chunk) and kept resident in two SBUF buffers
    (no reload for the residual add).
"""

import sys

sys.path.insert(0, "/opt/trn_rl_repo")

import numpy as np
import ml_dtypes

from contextlib import ExitStack

from concourse import bacc, bass, mybir, tile
from concourse.bass_utils import run_bass_kernel_spmd

F32 = mybir.dt.float32
F32R = mybir.dt.float32r
BF16 = mybir.dt.bfloat16
ADD = mybir.AluOpType.add
MULT = mybir.AluOpType.mult
SUB = mybir.AluOpType.subtract
BYPASS = mybir.AluOpType.bypass
AF = mybir.ActivationFunctionType

N_CORES = 8
N_IMG = 32
C = 64
H = W = 128
HP = WP = 130  # padded
RT = 4  # interior rows per tile
NT = H // RT  # 32 tiles, each [4, 128] = 512 psum columns
NHW = N_IMG * H * W  # global BN count
NL = 2 * H * W  # local elements per partition per conv (2 pairs x H x W / ... )
EPS = 1e-5

B_START = True


def _build_bass(n_cores=N_CORES, nhw=NHW):
    nc = bacc.Bacc(
        "TRN2", target_bir_lowering=False, debug=False, num_devices=n_cores
    )

    xs = nc.dram_tensor("xs", [4, C, H, W], BF16, kind="ExternalInput")
    w1l = nc.dram_tensor("w1l", [128, 9, C], BF16, kind="ExternalInput")
    w2l = nc.dram_tensor("w2l", [128, 9, C], BF16, kind="ExternalInput")
    gbd_t = nc.dram_tensor("gbd", [128, 4], F32, kind="ExternalInput")
    out = nc.dram_tensor("out", [4, C, H, W], F32, kind="ExternalOutput")

    rg8 = [list(range(n_cores))]

    with tile.TileContext(nc) as tc, ExitStack() as ctx:
        const = ctx.enter_context(tc.tile_pool(name="const", bufs=1))
        # big bf16 tag: out1 (padded) / out3 (unpadded) share slots
        bigp = ctx.enter_context(tc.tile_pool(name="bigp", bufs=3))
        xpadp = ctx.enter_context(tc.tile_pool(name="xpad", bufs=2))
        stage = ctx.enter_context(tc.tile_pool(name="stage", bufs=3))
        statp = ctx.enter_context(tc.tile_pool(name="stat", bufs=1))
        psum = ctx.enter_context(
            tc.tile_pool(name="psum", bufs=6, space="PSUM")
        )
        dram = ctx.enter_context(tc.tile_pool(name="dram", bufs=6, space="DRAM"))

        # ---- prologue: order DMAs so conv1 tile 0 unblocks earliest ----
        def ring_memset(t, hp, wp):
            """Zero only the 1-px padding ring of a [128, hp, wp] tile."""
            nc.gpsimd.memset(t[:, 0:1, :], 0.0)  # top row
            nc.gpsimd.memset(t[:, hp - 1 : hp, :], 0.0)  # bottom row
            nc.gpsimd.memset(t[:, 1 : hp - 1, 0:1], 0.0)  # left col
            nc.gpsimd.memset(t[:, 1 : hp - 1, wp - 1 : wp], 0.0)  # right col

        def ring_memset_v(t, hp, wp):
            """Ring memset on the (idle at prologue) vector engine."""
            nc.vector.memset(t[:, 0:1, :], 0.0)
            nc.vector.memset(t[:, hp - 1 : hp, :], 0.0)
            nc.vector.memset(t[:, 1 : hp - 1, 0:1], 0.0)
            nc.vector.memset(t[:, 1 : hp - 1, wp - 1 : wp], 0.0)

        w1_sb = const.tile([128, 9, C], BF16, tag="w1")
        w2_sb = const.tile([128, 9, C], BF16, tag="w2")
        nc.sync.dma_start(out=w1_sb[:], in_=w1l[:])

        # chunked interior loads on two independent DMA queues (the 256B-packet
        # scatter is packet-rate limited, not HBM limited): conv1 tile t only
        # waits for its chunk, and pair1 streams in parallel with pair0
        x_pad = {}
        gb = {}
        chunks0 = [(0, 8), (8, 24), (32, 32), (64, 32), (96, 32)]
        for p in (0, 1):
            xp = xpadp.tile([128, HP, WP], BF16, tag="xpad")
            ring_memset_v(xp, HP, WP)
            for ci, (r, nr) in enumerate(
                chunks0 if p == 0 else [(32 * c, 32) for c in range(4)]
            ):
                nc.sync.dma_start(
                    out=xp[:, 1 + r : 1 + r + nr, 1 : 1 + W],
                    in_=xs[2 * p : 2 * p + 2, :, r : r + nr, :],
                )
                if p == 0 and ci == 0:
                    # one packed BN-param load (g1,b1,g2,b2) right after the
                    # first small x chunk: the gamma^2 computes sit on the
                    # scalar queue IN ORDER ahead of all conv1 evictions, so
                    # this must land before the heavy chunk transfers
                    gball = const.tile([128, 4], F32, tag="gball")
                    nc.sync.dma_start(out=gball[:], in_=gbd_t[:])
                    for j, nm in enumerate(("g1", "b1", "g2", "b2")):
                        gb[nm] = gball[:, j : j + 1]
                    for nm in ("g1", "g2"):
                        ng = const.tile([128, 1], F32, tag=nm + "n")
                        nc.scalar.square(ng[:], gb[nm])
                        nc.scalar.mul(ng[:], ng[:], -1.0)
                        gb[nm + "n"] = ng[:]
            x_pad[p] = xp

        # lower-priority load goes behind both pairs' chunks
        nc.sync.dma_start(out=w2_sb[:], in_=w2l[:])

        # ---- warm up the collective stream (pays first-use setup during
        # conv1 instead of at the first real AllReduce) ----
        warm_in = dram.tile([128, 2], F32, tag="warm_in")
        warm_out = dram.tile([128, 2], F32, tag="warm_out")
        if not __import__("os").environ.get("KERNEL_NOCC"):
            nc.gpsimd.collective_compute(
                "AllReduce",
                ADD,
                replica_groups=rg8,
                ins=[warm_in[:].opt()],
                outs=[warm_out[:].opt()],
            )

        # per-tile bn_stats records: [128, NT*2 tiles, 6]
        st1 = statp.tile([128, 2 * NT, 6], F32, tag="st1")
        st2 = statp.tile([128, 2 * NT, 6], F32, tag="st2")

        def conv(w_sb, src, t):
            """9-tap conv for spatial tile t (both images). Returns psum tile."""
            ps = psum.tile([128, RT, W], F32, tag="ps")
            # tap-major, alternating quadrants: adjacent instructions hit
            # disjoint PE quadrants so they overlap
            for tap in range(9):
                for half in range(2):
                    ky, kx = tap // 3, tap % 3
                    lhsT = w_sb[64 * half : 64 * half + 64, tap, :]
                    rhs = src[
                        64 * half : 64 * half + 64,
                        RT * t + ky : RT * t + ky + RT,
                        kx : kx + W,
                    ]
                    nc.tensor.matmul(
                        ps[64 * half : 64 * half + 64, :, :],
                        lhsT,
                        rhs,
                        start=(tap == 0 and (half == 0 or B_START)),
                        stop=(tap == 8),
                        tile_position=(64 * half, 64 * half),
                    )
            return ps

        def stats_finalize(st, cc_name):
            """bn_aggr fold -> (mean, E[v^2]) chunks, fold halves, AllReduce.

            The AllReduce carries per-chunk (mean, E[v^2]); with all 16 chunks
            (2 partition halves x 8 cores) holding equal counts, bn_coeffs
            just divides the sums by 16.
            """
            mv = statp.tile([128, 2], F32, tag=cc_name + "mv")
            nc.vector.bn_aggr(mv[:], st[:])
            # overwrite var with mean^2 + var = E[v^2]
            nc.vector.scalar_tensor_tensor(
                mv[:, 1:2], mv[:, 0:1], mv[:, 0:1], mv[:, 1:2], MULT, ADD
            )
            cc_in = dram.tile([128, 2], F32, tag=cc_name + "in")
            cc_out = dram.tile([128, 2], F32, tag=cc_name + "out")
            nc.sync.dma_start(out=cc_in[:], in_=mv[:])
            if __import__("os").environ.get("KERNEL_NOCC"):
                nc.sync.dma_start(out=cc_out[:], in_=cc_in[:])
            else:
                nc.gpsimd.collective_compute(
                    "AllReduce",
                    ADD,
                    replica_groups=rg8,
                    ins=[cc_in[:].opt()],
                    outs=[cc_out[:].opt()],
                )
            # land each half duplicated on BOTH partition halves (four
            # parallel DMAs): the half-fold becomes a lane-aligned vector
            # add on all 128 partitions and s/b need no dup DMA afterward
            tot_a = statp.tile([128, 2], F32, tag=cc_name + "ta")
            tot_b = statp.tile([128, 2], F32, tag=cc_name + "tb")
            nc.sync.dma_start(out=tot_a[0:64, :], in_=cc_out[0:64, :])
            nc.gpsimd.dma_start(out=tot_a[64:128, :], in_=cc_out[0:64, :])
            nc.sync.dma_start(out=tot_b[0:64, :], in_=cc_out[64:128, :])
            nc.gpsimd.dma_start(out=tot_b[64:128, :], in_=cc_out[64:128, :])
            return tot_a, tot_b

        def bn_coeffs(tot, ng_sb, b_sb, nm):
            """tot[c] = 16-chunk sums of (mean, E[v^2]) -> s,b [128,1].

            negmean = -sum0/16 ; nvpe = negmean^2 - (sum1/16 + eps)
                    = -(var+eps) ; iv = -1/(var+eps)
            s = Sqrt((-gamma^2) * iv) = gamma/sqrt(var+eps)
            b = negmean*s + beta
            """
            m = statp.tile([128, 4], F32, tag=nm + "m")  # negmean, Ev2pe, nvpe, iv
            sfull = statp.tile([128, 1], F32, tag=nm + "s")
            bfull = statp.tile([128, 1], F32, tag=nm + "b")
            # fold the two partition halves (every partition has both)
            tot_a, tot_b = tot
            totf = statp.tile([128, 2], F32, tag=nm + "tf")
            nc.vector.tensor_tensor(totf[:], tot_a[:], tot_b[:], ADD)
            tot = totf
            nc.scalar.mul(m[:, 0:1], tot[:, 0:1], -1.0 / 16.0)  # negmean
            nc.vector.tensor_scalar(
                m[:, 1:2], tot[:, 1:2], 1.0 / 16.0, EPS, MULT, ADD
            )  # E[v^2] + eps
            nc.vector.scalar_tensor_tensor(
                m[:, 2:3], m[:, 0:1], m[:, 0:1], m[:, 1:2], MULT, SUB
            )  # -(var+eps)
            nc.vector.reciprocal(m[:, 3:4], m[:, 2:3])  # -1/(var+eps)
            nc.scalar.activation(
                sfull[:], m[:, 3:4], AF.Sqrt, 0.0, ng_sb
            )  # s = sqrt(gamma^2/(var+eps))
            nc.vector.scalar_tensor_tensor(
                bfull[:], m[:, 0:1], sfull[:], b_sb, MULT, ADD
            )  # b = beta - mean*s
            return sfull, bfull

        # ================= Phase A: conv1 on both pairs =================
        out1 = {}
        for p in (0, 1):
            o1 = bigp.tile([128, HP, WP], BF16, tag="big")
            ring_memset(o1, HP, WP)
            for t in range(NT):
                ps = conv(w1_sb, x_pad[p], t)
                col = NT * p + t
                # evict psum -> o1 (scalar) + stats (vector), one pass each
                nc.scalar.activation(
                    o1[:, 1 + RT * t : 1 + RT * t + RT, 1 : 1 + W],
                    ps[:],
                    AF.Copy,
                )
                nc.vector.bn_stats(
                    st1[:, col], ps[:].rearrange("p a b -> p (a b)")
                )
            out1[p] = o1

        tot1 = stats_finalize(st1, "cc1")
        s1, b1 = bn_coeffs(tot1, gb["g1n"], gb["b1"], "bn1")

        # ================= Phase B: bn1 + conv2 + residual ==============
        def apply_bn1(o1, r0, nr):
            """Apply bn1 affine in-place to interior rows [r0, r0+nr)."""
            nc.scalar.activation(
                o1[:, 1 + r0 : 1 + r0 + nr, 1 : 1 + W],
                o1[:, 1 + r0 : 1 + r0 + nr, 1 : 1 + W],
                AF.Identity,
                bias=b1[:],
                scale=s1[:],
            )

        out3 = {}
        for p in (0, 1):
            o1 = out1[p]
            apply_bn1(o1, 0, 2 * RT)
            o3 = bigp.tile([128, H, W], BF16, tag="big")
            for t in range(NT):
                # keep the affine 1-2 strips ahead of the conv reads,
                # 8 rows per instruction
                if t % 2 == 0 and RT * (2 + t) < H:
                    apply_bn1(o1, RT * (2 + t), 2 * RT)
                ps = conv(w2_sb, o1, t)
                col = NT * p + t
                # evict: o3 = psum + x (vector), stats on o3 (vector)
                nc.vector.tensor_tensor(
                    o3[:, RT * t : RT * t + RT, :],
                    ps[:],
                    x_pad[p][:, 1 + RT * t : 1 + RT * t + RT, 1 : 1 + W],
                    ADD,
                )
                nc.vector.bn_stats(
                    st2[:, col],
                    o3[:, RT * t : RT * t + RT, :].rearrange("p a b -> p (a b)"),
                )
            out3[p] = o3

        tot2 = stats_finalize(st2, "cc2")
        s2, b2 = bn_coeffs(tot2, gb["g2n"], gb["b2"], "bn2")

        # ================= Phase C: bn2 -> output =======================
        # stripes (small ones first so DMA ramps early); apply alternates
        # scalar/vector, DMA rotates over three queues.
        stripes = [(0, 8), (8, 8), (16, 16), (32, 16), (48, 16), (64, 16),
                   (80, 16), (96, 16), (112, 16)]
        queues = [nc.sync, nc.gpsimd]
        k = 0
        for p in (0, 1):
            o3 = out3[p]
            for r, nr in stripes:
                stg = stage.tile([128, 16, W], F32, tag="stg")
                if k % 2 == 0:
                    nc.scalar.activation(
                        stg[:, 0:nr, :],
                        o3[:, r : r + nr, :],
                        AF.Identity,
                        bias=b2[:],
                        scale=s2[:],
                    )
                else:
                    nc.vector.tensor_scalar(
                        stg[:, 0:nr, :],
                        o3[:, r : r + nr, :],
                        s2[:],
                        b2[:],
                        MULT,
                        ADD,
                    )
                queues[k % 2].dma_start(
                    out=out[2 * p : 2 * p + 2, :, r : r + nr, :],
                    in_=stg[:, 0:nr, :],
                )
                k += 1

    nc.finalize()
    return nc


_NC_CACHE = {}


def kernel(**inputs):
    x = np.asarray(inputs["x"], dtype=np.float32)
    w1 = np.asarray(inputs["w1"], dtype=np.float32)
    w2 = np.asarray(inputs["w2"], dtype=np.float32)
    g1 = np.asarray(inputs["bn1_gamma"], dtype=np.float32)
    b1 = np.asarray(inputs["bn1_beta"], dtype=np.float32)
    g2 = np.asarray(inputs["bn2_gamma"], dtype=np.float32)
    b2 = np.asarray(inputs["bn2_beta"], dtype=np.float32)

    if "nc" not in _NC_CACHE:
        _NC_CACHE["nc"] = _build_bass()
    nc = _NC_CACHE["nc"]

    # lhsT[i, tap, o] = w[o, i, ky, kx]; duplicated on both partition halves
    def pack(w, dt):
        wl = np.ascontiguousarray(w.transpose(1, 2, 3, 0).reshape(C, 9, C))
        return np.concatenate([wl, wl], axis=0).astype(dt)

    w1l = pack(w1, ml_dtypes.bfloat16)
    w2l = pack(w2, ml_dtypes.bfloat16)
    gbd = np.stack(
        [np.tile(g1, 2), np.tile(b1, 2), np.tile(g2, 2), np.tile(b2, 2)],
        axis=1,
    ).astype(np.float32)

    in_maps = []
    for k in range(N_CORES):
        in_maps.append(
            {
                "xs": np.ascontiguousarray(x[4 * k : 4 * k + 4]).astype(ml_dtypes.bfloat16),
                "w1l": w1l,
                "w2l": w2l,
                "gbd": gbd,
            }
        )

    trace = bool(int(__import__("os").environ.get("KERNEL_TRACE", "0")))
    res = run_bass_kernel_spmd(
        nc, in_maps, core_ids=list(range(N_CORES)), trace=trace
    )
    if trace:
        kernel.last_exec_time_ns = res.exec_time_ns
        kernel.last_results = res
    out = np.concatenate([r["out"] for r in res.results], axis=0)
    return out.astype(np.float32)


if __name__ == "__main__":
    nc = _build_bass()
    print("build ok")
